# revision 2
# baseline (speedup 1.0000x reference)
"""HGCN decoder kernel for Trainium2 (8 NeuronCores, SPMD) — v2.

Pipeline (matches the HGCN decoder reference):
  1. HypLinear: mv = proj(mobius_matvec(W, x)); h = proj(mobius_add(mv, hyp_bias))
  2. HypAgg:    xt = logmap0(h); agg = segment_sum(edge_w * xt[src], dst); h = proj(expmap0(agg))
  3. HypAct + decode: logmap0(proj(expmap0(logmap0(h))))

v2 design:
  - Global in-degree sort relabels nodes so each 1024-rank band maps to one
    (tile, all-cores) stripe -> tight per-tile slot widths for phase B.
  - Phase A (node-sharded): host feeds x pre-transposed as bf16 [4,128,12544]
    slabs; matmuls run straight off SBUF (no on-chip transposes). ||x||^2 via
    squared-slab x ones matmul. Pointwise tail batched over all tiles.
  - Phase B (dst-sharded): xt table packed 4 nodes/row ([25088, 64] f32);
    InstDMAGatherAnt gathers NI slots per instruction (int16 quad indices);
    DVE applies host-built per-slot phase masks (weight in the mask) and
    segment-reduces per dst tile; then the pointwise exp/log tail.
"""

import sys

sys.path.insert(0, "/opt/trn_rl_repo")

import numpy as np

import concourse.bass as bass
import concourse.mybir as mybir
from concourse.bass_utils import run_bass_kernel_spmd
from concourse.tile import TileContext

F32 = mybir.dt.float32
BF16 = mybir.dt.bfloat16
I16 = mybir.dt.int16
I32 = mybir.dt.int32


# The pinned walrus build rejects InstDrain with more than one or two sem
# waits ("Too many sync wait commands"). Split the TileContext tail drain's
# waits across a chain of single-wait drains instead.
def _patched_drain_and_barrier(self, tick_clock, wait_clock):
    from concourse.vector_clock import ScopedClock

    drain_inst = self.nc.sync.drain()
    wait_clock.add_sem_waits(
        drain_inst.ins, ScopedClock({None: tick_clock.global_clock})
    )
    si = drain_inst.ins.sync_info
    if si is not None and len(si.on_wait) > 1:
        extras = list(si.on_wait[1:])
        del si.on_wait[1:]
        for w in extras:
            d = self.nc.sync.drain()
            dsi = d.ins.sync_info
            if dsi is None:
                d.ins.sync_info = mybir.SyncInfo(on_wait=[w], on_update=[])
            else:
                dsi.on_wait.append(w)

    self.nc.all_engine_barrier()
    assert self.sems is not None
    popped = self.nc._tile_sem_poison_stack.pop()
    assert popped is self._sem_poison
    self.nc.clear_and_free_semaphores(list(self.sems.allocated().values()))
    self.nc.all_engine_barrier()


TileContext._drain_and_barrier = _patched_drain_and_barrier


def _split_multi_waits(nc):
    """Walrus here allows at most one sem wait per instruction; hoist extras
    onto no-fuse NOPs inserted immediately before the instruction."""
    for f in nc.m.functions:
        for blk in f.blocks:
            i = 0
            while i < len(blk.instructions):
                inst = blk.instructions[i]
                si = inst.sync_info
                if si is not None and len(si.on_wait) > 1:
                    extras = list(si.on_wait[:-1])
                    si.on_wait = [si.on_wait[-1]]
                    for w in extras:
                        ni = nc.engines[inst.engine].nop(nofuse=True).ins
                        removed = False
                        for f2 in nc.m.functions:
                            for b2 in f2.blocks:
                                for j in range(len(b2.instructions) - 1, -1, -1):
                                    if b2.instructions[j] is ni:
                                        del b2.instructions[j]
                                        removed = True
                                        break
                                if removed:
                                    break
                            if removed:
                                break
                        assert removed, "appended nop not found"
                        ni.sync_info = mybir.SyncInfo(on_wait=[w], on_update=[])
                        blk.instructions.insert(i, ni)
                        i += 1
                i += 1


def _insert_library_loads(nc):
    from concourse.library_config import all_libraries, standard
    import bass_rust as _bass_rust

    mask = {}
    for lib in all_libraries:
        for t in lib.instructions:
            mask[t] = mask.get(t, 0) | (1 << lib.index)
    _bass_rust.insert_library_loads(nc, mask, len(all_libraries), standard.index)
    mybir.codegen_inst_isa_subclasses(nc)


ALU = mybir.AluOpType
ACT = mybir.ActivationFunctionType
AX = mybir.AxisListType


def _register_consts(nc, values):
    for v in values:
        v = float(v)
        if (F32, v) in nc.const_aps.aps:
            continue
        t = nc.alloc_sbuf_tensor(f"const-f32-{v}", [128, 1], F32)
        nc.vector.memset(t.ap(), v)
        nc.const_aps.aps[(F32, v)] = t.ap()

N = 100000
D = 512
K = 16
NC = 8
P = 128
T = 98                 # tiles per core
NP = T * P             # 12544 padded nodes per core
NTOT = NC * NP         # 100352
NQUAD = NTOT // 4      # 25088 table rows (4 nodes packed per row)
ES = 64                # table row elements (f32) = 256B
CH = D // P            # 4 contraction chunks

NI = 1024              # slots per dma_gather call (tuned by ladder bench)
NQ = 4                 # SWDGE queues (descgen parallelism, ucode max 4)

MAXN = np.float32(1.0 - 4e-3)   # (1 - BALL_EPS) / sqrt(c)
MIN_N2 = np.float32(1e-30)      # MIN_NORM**2

_CACHE = {}
_BUILD_CONSTS = {"y2": 0.0}


# ---------------------------------------------------------------- phase A ---
def build_phase_a():
    nc = bass.Bass()
    _register_consts(nc, [float(MIN_N2)])
    x_in = nc.dram_tensor("xT", [CH, P, NP], BF16, kind="ExternalInput")
    wt_in = nc.dram_tensor("wT", [P, CH, K], BF16, kind="ExternalInput")
    hb_in = nc.dram_tensor("hb", [P, K], F32, kind="ExternalInput")
    xt_out = nc.dram_tensor("xt", [NP, K], F32, kind="ExternalOutput")

    y2f = float(_BUILD_CONSTS["y2"])

    with TileContext(nc) as tc:
        with (
            tc.tile_pool(name="persist", bufs=1) as pp,
            tc.tile_pool(name="stream", bufs=4) as sp,
            tc.tile_pool(name="psmv", bufs=1, space="PSUM") as psmv,
            tc.tile_pool(name="psxn", bufs=1, space="PSUM") as psxn,
        ):
            wt_sb = pp.tile([P, CH, K], BF16)
            nc.sync.dma_start(wt_sb[:], wt_in[:, :, :])
            hb_sb = pp.tile([P, K], F32)
            nc.sync.dma_start(hb_sb[:], hb_in[:, :])
            ones_sb = pp.tile([P, 1], BF16)
            nc.vector.memset(ones_sb[:], 1.0)

            mv_ps = psmv.tile([P, T, K], F32)     # 4 PSUM banks (98*16*4B)
            xn2_ps = psxn.tile([P, T], F32)       # 1 PSUM bank

            # two half-slabs so tiles 0-48 start as soon as the first half
            # of each chunk lands
            TH = T // 2                     # 49 tiles in the first half
            NH = TH * P
            xsa = pp.tile([P, CH, NH], BF16)
            xsb = pp.tile([P, CH, NP - NH], BF16)
            for c in range(CH):
                nc.sync.dma_start(xsa[:, c, :], x_in[c, :, 0:NH])
            for c in range(CH):
                nc.sync.dma_start(xsb[:, c, :], x_in[c, :, NH:])

            for t in range(T):
                slab = xsa if t < TH else xsb
                o = t * P if t < TH else (t - TH) * P
                sq_t = sp.tile([P, CH, P], BF16, tag="sqt")
                if t % 2 == 0:
                    nc.scalar.activation(
                        sq_t[:], slab[:, :, o:o + P], ACT.Square
                    )
                else:
                    nc.vector.tensor_tensor(
                        sq_t[:], slab[:, :, o:o + P],
                        slab[:, :, o:o + P], ALU.mult,
                    )
                for c in range(CH):
                    nc.tensor.matmul(
                        mv_ps[:, t, :], lhsT=slab[:, c, o:o + P],
                        rhs=wt_sb[:, c, :], start=(c == 0), stop=(c == CH - 1),
                    )
                for c in range(CH):
                    nc.tensor.matmul(
                        xn2_ps[:, t:t + 1], lhsT=sq_t[:, c, :],
                        rhs=ones_sb[:], start=(c == 0), stop=(c == CH - 1),
                    )

            mx_all = pp.tile([P, T, K], F32)
            nc.scalar.copy(mx_all[:], mv_ps[:])
            xn2_all = pp.tile([P, T], F32)
            nc.vector.tensor_copy(xn2_all[:], xn2_ps[:])

            # ---------------- batched pointwise (all tiles at once) --------
            def bcast(col):
                return col[:, :, None].to_broadcast([P, T, K])

            s1 = pp.tile([P, T], F32)    # xn
            nc.scalar.activation(s1[:], xn2_all[:], ACT.Sqrt, bias=float(MIN_N2))
            # artanh(xn) = 0.5*(ln(1+xn) - ln(1-xn))
            lu = pp.tile([P, T], F32)
            nc.scalar.activation(lu[:], s1[:], ACT.Ln, bias=1.0, scale=1.0)
            lv = pp.tile([P, T], F32)
            nc.scalar.activation(lv[:], s1[:], ACT.Ln, bias=1.0, scale=-1.0)
            at = pp.tile([P, T], F32)
            nc.vector.tensor_tensor(at[:], lu[:], lv[:], ALU.subtract)
            nc.vector.tensor_scalar_mul(at[:], at[:], 0.5)
            rxn = pp.tile([P, T], F32)
            nc.vector.reciprocal(rxn[:], s1[:])
            s_fac = pp.tile([P, T], F32)   # artanh(xn)/xn
            nc.vector.tensor_tensor(s_fac[:], at[:], rxn[:], ALU.mult)

            mxn2 = pp.tile([P, T], F32)
            sq16 = sp.tile([P, T, K], F32, tag="sq16")
            nc.vector.tensor_tensor(sq16[:], mx_all[:], mx_all[:], ALU.mult)
            nc.vector.tensor_reduce(mxn2[:], sq16[:], axis=AX.X, op=ALU.add)
            mxn = pp.tile([P, T], F32)
            nc.scalar.activation(mxn[:], mxn2[:], ACT.Sqrt, bias=float(MIN_N2))

            z = pp.tile([P, T], F32)
            nc.vector.tensor_tensor(z[:], mxn[:], s_fac[:], ALU.mult)
            tt = pp.tile([P, T], F32)     # tanh(mxn/xn * artanh(xn)) = ||mv||
            nc.scalar.activation(tt[:], z[:], ACT.Tanh)
            # proj(mv): scale mx by min(tt, MAXN)/mxn
            tm = pp.tile([P, T], F32)
            nc.vector.tensor_scalar(tm[:], tt[:], float(MAXN), None, ALU.min)
            rmxn = pp.tile([P, T], F32)
            nc.vector.reciprocal(rmxn[:], mxn[:])
            gsc = pp.tile([P, T], F32)
            nc.vector.tensor_tensor(gsc[:], tm[:], rmxn[:], ALU.mult)
            mv = mx_all  # in-place: mv = mx * gsc
            nc.vector.tensor_tensor(mv[:], mx_all[:], bcast(gsc), ALU.mult)

            # mobius_add(mv, hb):  x2 = tm^2, y2 = const, xy = <mv, hb>
            x2 = pp.tile([P, T], F32)
            nc.scalar.activation(x2[:], tm[:], ACT.Square)
            xyp = sp.tile([P, T, K], F32, tag="xyp")
            nc.vector.tensor_tensor(
                xyp[:], mv[:], hb_sb[:, None, :].to_broadcast([P, T, K]), ALU.mult
            )
            xy = pp.tile([P, T], F32)
            nc.vector.tensor_reduce(xy[:], xyp[:], axis=AX.X, op=ALU.add)

            coefA = pp.tile([P, T], F32)   # 1 + 2*xy + y2
            nc.vector.tensor_scalar(coefA[:], xy[:], 2.0, 1.0 + y2f, ALU.mult, ALU.add)
            coefB = pp.tile([P, T], F32)   # 1 - x2
            nc.vector.tensor_scalar(coefB[:], x2[:], -1.0, 1.0, ALU.mult, ALU.add)
            den = pp.tile([P, T], F32)     # 1 + 2*xy + x2*y2
            nc.vector.tensor_scalar(den[:], x2[:], y2f, None, ALU.mult)
            nc.vector.tensor_scalar(den[:], den[:], 1.0, None, ALU.add)
            tmp2 = pp.tile([P, T], F32)
            nc.vector.tensor_scalar(tmp2[:], xy[:], 2.0, None, ALU.mult)
            nc.vector.tensor_tensor(den[:], den[:], tmp2[:], ALU.add)
            nc.vector.tensor_scalar(den[:], den[:], 1e-15, None, ALU.max)
            rden = pp.tile([P, T], F32)
            nc.vector.reciprocal(rden[:], den[:])

            hterm = sp.tile([P, T, K], F32, tag="hterm")
            nc.vector.tensor_tensor(
                hterm[:], hb_sb[:, None, :].to_broadcast([P, T, K]), bcast(coefB),
                ALU.mult,
            )
            h = mv  # in-place
            nc.vector.tensor_tensor(h[:], mv[:], bcast(coefA), ALU.mult)
            nc.vector.tensor_tensor(h[:], h[:], hterm[:], ALU.add)
            nc.vector.tensor_tensor(h[:], h[:], bcast(rden), ALU.mult)

            # proj(h)
            hn2 = pp.tile([P, T], F32)
            nc.vector.tensor_tensor(sq16[:], h[:], h[:], ALU.mult)
            nc.vector.tensor_reduce(hn2[:], sq16[:], axis=AX.X, op=ALU.add)
            hn = pp.tile([P, T], F32)
            nc.scalar.activation(hn[:], hn2[:], ACT.Sqrt, bias=float(MIN_N2))
            rhn = pp.tile([P, T], F32)
            nc.vector.reciprocal(rhn[:], hn[:])
            pf = pp.tile([P, T], F32)
            nc.vector.tensor_scalar(pf[:], rhn[:], float(MAXN), 1.0, ALU.mult, ALU.min)
            nc.vector.tensor_tensor(h[:], h[:], bcast(pf), ALU.mult)
            hnp = pp.tile([P, T], F32)     # ||proj(h)|| = min(hn, MAXN)
            nc.vector.tensor_scalar(hnp[:], hn[:], float(MAXN), None, ALU.min)

            # xt = logmap0(h) = artanh(hnp) * h / hnp
            nc.scalar.activation(lu[:], hnp[:], ACT.Ln, bias=1.0, scale=1.0)
            nc.scalar.activation(lv[:], hnp[:], ACT.Ln, bias=1.0, scale=-1.0)
            nc.vector.tensor_tensor(at[:], lu[:], lv[:], ALU.subtract)
            nc.vector.tensor_scalar_mul(at[:], at[:], 0.5)
            nc.vector.tensor_scalar(hnp[:], hnp[:], 1e-15, None, ALU.max)
            nc.vector.reciprocal(rhn[:], hnp[:])
            nc.vector.tensor_tensor(at[:], at[:], rhn[:], ALU.mult)
            nc.vector.tensor_tensor(h[:], h[:], bcast(at), ALU.mult)

            nc.sync.dma_start(xt_out.rearrange("(t p) k -> p t k", p=P), h[:])
    _split_multi_waits(nc)
    return nc


# ---------------------------------------------------------------- phase B ---
def build_phase_b(n_calls, segments):
    """segments: per call, list of (tile, lo, hi, first) with 0<=lo<hi<=NI/128
    covering the call's slot columns."""
    nc = bass.Bass(num_swdge_queues=NQ)
    _register_consts(nc, [float(MIN_N2)])
    CPC = NI // 128                      # columns per call
    ICOLS = NI // 16                     # idx sbuf cols per call
    tbl = nc.dram_tensor("tbl", [NQUAD, ES], F32, kind="ExternalInput")
    idx_in = nc.dram_tensor("idx", [P, n_calls * ICOLS], I16,
                            kind="ExternalInput")
    msk_in = nc.dram_tensor("msk", [P, n_calls * CPC, 4], F32,
                            kind="ExternalInput")
    out_d = nc.dram_tensor("out", [NP, K], F32, kind="ExternalOutput")

    with TileContext(nc) as tc:
        with (
            tc.tile_pool(name="persist", bufs=1) as pp,
            tc.tile_pool(name="stream", bufs=2 * NQ) as sp,
            tc.tile_pool(name="small", bufs=2 * NQ) as sm,
        ):
            agg = pp.tile([P, T, K], F32)
            nc.vector.memset(agg[:], 0.0)
            nireg = nc.gpsimd.to_reg(NI)

            for g in range(n_calls):
                idx_sb = sm.tile([P, ICOLS], I16, tag="idx")
                nc.sync.dma_start(idx_sb[:], idx_in[:, g * ICOLS:(g + 1) * ICOLS])
                msk_sb = sm.tile([P, CPC, 4], F32, tag="msk")
                nc.sync.dma_start(msk_sb[:], msk_in[:, g * CPC:(g + 1) * CPC, :])
                gbuf = sp.tile([P, CPC, 4, K], F32, tag="g")
                nc.gpsimd.dma_gather(
                    out_ap=gbuf[:].rearrange("p c f k -> p c (f k)"),
                    in_ap=tbl[:, :],
                    idxs_ap=idx_sb[:],
                    num_idxs=NI,
                    num_idxs_reg=nireg,
                    elem_size=ES,
                    queue_num=g % NQ,
                )
                # weight+phase select: gbuf *= mask (broadcast over k)
                nc.vector.tensor_tensor(
                    gbuf[:], gbuf[:],
                    msk_sb[:, :, :, None].to_broadcast([P, CPC, 4, K]),
                    ALU.mult,
                )
                # phase tree-sum: [P, CPC, 4, K] -> sel [P, CPC, K]
                t2 = sm.tile([P, CPC, 2, K], F32, tag="t2")
                nc.vector.tensor_tensor(
                    t2[:], gbuf[:, :, 0:2, :], gbuf[:, :, 2:4, :], ALU.add
                )
                sel = sm.tile([P, CPC, K], F32, tag="sel")
                nc.vector.tensor_tensor(
                    sel[:], t2[:, :, 0, :], t2[:, :, 1, :], ALU.add
                )
                for (t, lo, hi, first) in segments[g]:
                    src = sel[:, lo:hi, :].rearrange("p c k -> p k c")
                    if first:
                        nc.vector.tensor_reduce(
                            agg[:, t, :], src, axis=AX.X, op=ALU.add
                        )
                    else:
                        tmp = sm.tile([P, K], F32, tag="tmp")
                        nc.vector.tensor_reduce(tmp[:], src, axis=AX.X,
                                                op=ALU.add)
                        nc.vector.tensor_tensor(agg[:, t, :], agg[:, t, :],
                                                tmp[:], ALU.add)

            # -------- pointwise tail: out = logmap0(proj(expmap0(logmap0(
            #          proj(expmap0(agg))))))
            def bcast(col):
                return col[:, :, None].to_broadcast([P, T, K])

            sq16 = sp.tile([P, T, K], F32, tag="sq16")
            an2 = pp.tile([P, T], F32)
            nc.vector.tensor_tensor(sq16[:], agg[:], agg[:], ALU.mult)
            nc.vector.tensor_reduce(an2[:], sq16[:], axis=AX.X, op=ALU.add)
            an = pp.tile([P, T], F32)
            nc.scalar.activation(an[:], an2[:], ACT.Sqrt, bias=float(MIN_N2))
            te = pp.tile([P, T], F32)
            nc.scalar.activation(te[:], an[:], ACT.Tanh)
            ran = pp.tile([P, T], F32)
            nc.vector.reciprocal(ran[:], an[:])
            er = pp.tile([P, T], F32)
            nc.vector.tensor_tensor(er[:], te[:], ran[:], ALU.mult)
            h = agg
            nc.vector.tensor_tensor(h[:], agg[:], bcast(er), ALU.mult)
            # proj: factor min(1, MAXN/te); ||h|| = te
            rte = pp.tile([P, T], F32)
            nc.vector.tensor_scalar(rte[:], te[:], 1e-15, None, ALU.max)
            nc.vector.reciprocal(rte[:], rte[:])
            pf = pp.tile([P, T], F32)
            nc.vector.tensor_scalar(pf[:], rte[:], float(MAXN), 1.0, ALU.mult, ALU.min)
            nc.vector.tensor_tensor(h[:], h[:], bcast(pf), ALU.mult)
            hpn = pp.tile([P, T], F32)
            nc.vector.tensor_scalar(hpn[:], te[:], float(MAXN), None, ALU.min)

            # xt2 = logmap0(h): ratio = artanh(hpn)/max(hpn, eps)
            lu = pp.tile([P, T], F32)
            lv = pp.tile([P, T], F32)
            nc.scalar.activation(lu[:], hpn[:], ACT.Ln, bias=1.0, scale=1.0)
            nc.scalar.activation(lv[:], hpn[:], ACT.Ln, bias=1.0, scale=-1.0)
            at2 = pp.tile([P, T], F32)      # artanh(hpn) = ||xt2||
            nc.vector.tensor_tensor(at2[:], lu[:], lv[:], ALU.subtract)
            nc.vector.tensor_scalar_mul(at2[:], at2[:], 0.5)
            tmp = pp.tile([P, T], F32)
            nc.vector.tensor_scalar(tmp[:], hpn[:], 1e-15, None, ALU.max)
            nc.vector.reciprocal(tmp[:], tmp[:])
            nc.vector.tensor_tensor(tmp[:], at2[:], tmp[:], ALU.mult)
            nc.vector.tensor_tensor(h[:], h[:], bcast(tmp), ALU.mult)

            # h2 = proj(expmap0(xt2)); ||xt2|| = at2 (clip for reciprocal)
            te2 = pp.tile([P, T], F32)
            nc.scalar.activation(te2[:], at2[:], ACT.Tanh)
            nc.vector.tensor_scalar(tmp[:], at2[:], 1e-15, None, ALU.max)
            nc.vector.reciprocal(tmp[:], tmp[:])
            nc.vector.tensor_tensor(tmp[:], te2[:], tmp[:], ALU.mult)
            nc.vector.tensor_tensor(h[:], h[:], bcast(tmp), ALU.mult)
            nc.vector.tensor_scalar(rte[:], te2[:], 1e-15, None, ALU.max)
            nc.vector.reciprocal(rte[:], rte[:])
            nc.vector.tensor_scalar(pf[:], rte[:], float(MAXN), 1.0, ALU.mult, ALU.min)
            nc.vector.tensor_tensor(h[:], h[:], bcast(pf), ALU.mult)
            hpn2 = pp.tile([P, T], F32)
            nc.vector.tensor_scalar(hpn2[:], te2[:], float(MAXN), None, ALU.min)

            # out = logmap0(h2)
            nc.scalar.activation(lu[:], hpn2[:], ACT.Ln, bias=1.0, scale=1.0)
            nc.scalar.activation(lv[:], hpn2[:], ACT.Ln, bias=1.0, scale=-1.0)
            nc.vector.tensor_tensor(at2[:], lu[:], lv[:], ALU.subtract)
            nc.vector.tensor_scalar_mul(at2[:], at2[:], 0.5)
            nc.vector.tensor_scalar(tmp[:], hpn2[:], 1e-15, None, ALU.max)
            nc.vector.reciprocal(tmp[:], tmp[:])
            nc.vector.tensor_tensor(tmp[:], at2[:], tmp[:], ALU.mult)
            nc.vector.tensor_tensor(h[:], h[:], bcast(tmp), ALU.mult)

            nc.sync.dma_start(out_d.rearrange("(t p) k -> p t k", p=P), h[:])
    _split_multi_waits(nc)
    _insert_library_loads(nc)
    return nc


# ------------------------------------------------------------------- host ---
def _hyp_bias(bias):
    b = bias.astype(np.float64)
    bn = max(np.sqrt((b * b).sum()), 1e-15)
    hb = np.tanh(bn) * b / bn
    n = max(np.sqrt((hb * hb).sum()), 1e-15)
    if n > float(MAXN):
        hb = hb / n * float(MAXN)
    return hb.astype(np.float32)


def _relabel(edge_dst):
    """Global in-degree sort. Returns order (rank -> node) and the
    position map (rank -> table position)."""
    deg = np.bincount(edge_dst, minlength=N)
    order = np.argsort(-deg, kind="stable")          # rank -> node
    # rank r -> (core, tile, part): t = r//1024, c = (r//128)%8, p = r%128
    r = np.arange(NTOT)
    pos_of_rank = ((r // P) % NC) * NP + (r // (P * NC)) * P + r % P
    return deg, order, pos_of_rank


def _prep_phase_b(edge_w, edge_src, edge_dst, rank_of, pos_of_rank):
    """Build slot grid: per core idx (quad, int16) + phase masks + segments."""
    src_rank = rank_of[edge_src]
    dst_rank = rank_of[edge_dst]
    src_pos = pos_of_rank[src_rank]
    dst_pos = pos_of_rank[dst_rank]

    dst_core = dst_pos // NP
    loc = dst_pos % NP
    dst_t = loc // P
    dst_p = loc % P

    # per-tile slot width: max slots over (core, part) per tile
    cnt = np.zeros((NC, T, P), dtype=np.int64)
    np.add.at(cnt, (dst_core, dst_t, dst_p), 1)
    md_list = cnt.max(axis=(0, 2)).astype(np.int64)
    md_list = np.maximum(md_list, 1)
    base = np.concatenate([[0], np.cumsum(md_list)]).astype(np.int64)
    S = int(base[-1])
    CPC = NI // 128
    n_calls = (S + CPC - 1) // CPC
    S_pad = n_calls * CPC

    idx16 = np.zeros((NC, P, S_pad), dtype=np.int16)
    mask = np.zeros((NC, P, S_pad, 4), dtype=np.float32)

    # slot rank within (core, tile, part) via stable sort
    key = (dst_core * T + dst_t) * P + dst_p
    so = np.argsort(key, kind="stable")
    key_s = key[so]
    uniq, first_pos = np.unique(key_s, return_index=True)
    slot_within = np.arange(len(key_s)) - np.repeat(first_pos,
                                                    np.diff(np.concatenate(
                                                        [first_pos, [len(key_s)]])))
    cc = dst_core[so]
    tt = dst_t[so]
    pp_ = dst_p[so]
    col = base[tt] + slot_within
    sp_ = src_pos[so]
    idx16[cc, pp_, col] = (sp_ // 4).astype(np.int16)
    mask[cc, pp_, col, sp_ % 4] = edge_w[so]

    # segments per call: tiles overlapping [g*CPC, (g+1)*CPC)
    segments = []
    tile_first_seen = set()
    tile_lo = base[:-1].copy()
    tile_hi = base[1:].copy()
    tile_hi[-1] = S_pad
    for g in range(n_calls):
        g0, g1 = g * CPC, (g + 1) * CPC
        segs = []
        ts = np.nonzero((tile_lo < g1) & (tile_hi > g0))[0]
        for t in ts:
            lo = max(int(tile_lo[t]), g0) - g0
            hi = min(int(tile_hi[t]), g1) - g0
            first = t not in tile_first_seen
            tile_first_seen.add(t)
            segs.append((int(t), lo, hi, first))
        segments.append(segs)

    # wrap idx into the 16-partition, 8x-replicated ant layout per call:
    # flat[i] = slot (p=i%128, col=g*CPC + i//128); wrapped[pp, s]=flat[s*16+pp]
    idx_ant = np.zeros((NC, P, n_calls * (NI // 16)), dtype=np.int16)
    for c in range(NC):
        a = idx16[c].T.reshape(n_calls, NI // 16, 16)      # [g, s, pp]
        w16 = a.transpose(2, 0, 1).reshape(16, -1)         # [16, g*(NI/16)]
        idx_ant[c] = np.tile(w16, (8, 1))
    return md_list, S, n_calls, segments, idx_ant, mask


def kernel(x, weight, bias, edge_w, edge_src, edge_dst):
    x = np.asarray(x, dtype=np.float32)
    weight = np.asarray(weight, dtype=np.float32)
    bias = np.asarray(bias, dtype=np.float32)
    edge_w = np.asarray(edge_w, dtype=np.float32)
    edge_src = np.asarray(edge_src).astype(np.int64)
    edge_dst = np.asarray(edge_dst).astype(np.int64)
    import ml_dtypes

    hb = _hyp_bias(bias)
    _BUILD_CONSTS["y2"] = float((hb.astype(np.float64) ** 2).sum())

    deg, order, pos_of_rank = _relabel(edge_dst)
    rank_of = np.empty(N, dtype=np.int64)
    rank_of[order] = np.arange(N)

    # position -> rank (for phase A input build and final unshuffle)
    rank_of_pos = np.empty(NTOT, dtype=np.int64)
    rank_of_pos[pos_of_rank] = np.arange(NTOT)

    # ---- phase A input prep ----
    xb = x.astype(ml_dtypes.bfloat16)
    wT = weight.T.reshape(CH, P, K).transpose(1, 0, 2)     # [128, 4, 16]
    wT = np.ascontiguousarray(wT).astype(ml_dtypes.bfloat16)
    hb_rep = np.tile(hb[None, :], (P, 1))

    if "A" not in _CACHE or _CACHE.get("A_y2") != _BUILD_CONSTS["y2"]:
        _CACHE["A"] = build_phase_a()
        _CACHE["A_y2"] = _BUILD_CONSTS["y2"]
    nc_a = _CACHE["A"]

    in_maps_a = []
    for c in range(NC):
        ranks_c = rank_of_pos[c * NP:(c + 1) * NP]         # local (t,p) order
        node_c = np.where(ranks_c < N, order[np.minimum(ranks_c, N - 1)],
                          order[0])
        xs = xb[node_c]                                    # [12544, 512] bf16
        xT = np.ascontiguousarray(xs.T.reshape(CH, P, NP))
        in_maps_a.append({"xT": xT, "wT": wT, "hb": hb_rep})

    res_a = run_bass_kernel_spmd(
        nc_a, in_maps_a, core_ids=list(range(NC)), **_CACHE.get("run_kwargs", {})
    )
    xt_full = np.concatenate([res_a.results[c]["xt"] for c in range(NC)], axis=0)
    _CACHE["last_exec_a"] = res_a.exec_time_ns

    # ---- phase B ----
    sig = (edge_src.shape[0], int(edge_src[:64].sum()), int(edge_dst[:64].sum()),
           float(edge_w[:64].sum()))
    if _CACHE.get("Bprep_sig") != sig:
        _CACHE["Bprep"] = _prep_phase_b(edge_w, edge_src, edge_dst,
                                        rank_of, pos_of_rank)
        _CACHE["Bprep_sig"] = sig
    md_list, S, n_calls, segments, idx_ant, mask = _CACHE["Bprep"]

    key_b = ("B", n_calls, tuple(md_list))
    if key_b not in _CACHE:
        _CACHE[key_b] = build_phase_b(n_calls, segments)
    nc_b = _CACHE[key_b]

    tblq = np.ascontiguousarray(xt_full.reshape(NQUAD, ES))
    in_maps_b = [
        {"tbl": tblq, "idx": idx_ant[c], "msk": mask[c]} for c in range(NC)
    ]
    res_b = run_bass_kernel_spmd(
        nc_b, in_maps_b, core_ids=list(range(NC)), **_CACHE.get("run_kwargs", {})
    )
    _CACHE["last_exec_b"] = res_b.exec_time_ns

    # ---- unshard: out position (c, t, p) -> original node ----
    out = np.empty((N, K), dtype=np.float32)
    allpos = np.concatenate([res_b.results[c]["out"] for c in range(NC)],
                            axis=0)                        # [NTOT, K] pos order
    ranks_all = rank_of_pos
    real = ranks_all < N
    out[order[ranks_all[real]]] = allpos[real]
    return out
